# revision 2
# baseline (speedup 1.0000x reference)
"""Segment-mean (sentence pooling) Bass/Tile kernel for Trainium2.

Problem: last_hidden_state [16, 4096, 1024] f32, sentence_mask [16, 4096] int,
num_sents=32. For each (batch, sentence id): mean of hidden states at seq
positions whose mask equals the id. Returns (embeddings [16, 32, 1024] f32,
unique_sents [32] int).

Strategy: data-parallel over batch across 8 NeuronCores (2 batches/core).
On each core, per batch: stream 32 seq-chunks of [128, 1024] f32 hidden via
DMA; matmul each against a host-precomputed one-hot mask chunk [128, 32]
(pre-scaled by 1/count so PSUM directly accumulates the mean) as the
stationary lhsT, accumulating into PSUM [32, 512] x2; copy PSUM -> SBUF and
DMA out. Memory-bound: 32 MiB/core of hidden states.
"""

import numpy as np

BATCH, SEQ, HID, NS = 16, 4096, 1024, 32
P = 128
NCORES = 8
BPC = BATCH // NCORES  # batches per core
CHUNKS = SEQ // P  # seq chunks of 128

# "f32" = exact fp32 matmul (4 PE cycles/row); "f32r" = fp32r matmul
# (1 cycle/row, ~1e-4 relative error, inputs DMA-cast to fp32r).
MODE = "f32"
HBUFS = 8  # hidden-tile double-buffer depth

_nc_cache = {}


def _build_nc(mode):
    import concourse.mybir as mybir
    import concourse.tile as tile
    from concourse import bacc

    nc = bacc.Bacc("TRN2", target_bir_lowering=False, debug=False)
    h = nc.dram_tensor("h", [BPC, SEQ, HID], mybir.dt.float32, kind="ExternalInput")
    w = nc.dram_tensor(
        "w", [BPC, P, CHUNKS, NS], mybir.dt.float32, kind="ExternalInput"
    )
    o = nc.dram_tensor("o", [BPC, NS, HID], mybir.dt.float32, kind="ExternalOutput")
    h4 = h.ap().rearrange("b (c p) d -> b c p d", p=P)

    sb_dt = mybir.dt.float32 if mode == "f32" else mybir.dt.float32r

    with tile.TileContext(nc) as tc:
        with (
            tc.tile_pool(name="wp", bufs=1) as wp,
            tc.tile_pool(name="hp", bufs=HBUFS) as hp,
            tc.tile_pool(name="op", bufs=2) as op,
            tc.tile_pool(name="pp", bufs=2 * BPC, space="PSUM") as pp,
        ):
            wts = []
            for b in range(BPC):
                wt = wp.tile([P, CHUNKS, NS], sb_dt, tag=f"w{b}")
                if mode == "f32":
                    nc.sync.dma_start(wt[:], w.ap()[b])
                else:
                    nc.gpsimd.dma_start(wt[:], w.ap()[b])  # casts f32 -> f32r
                wts.append(wt)
            for b in range(BPC):
                ps = [
                    pp.tile([NS, 512], mybir.dt.float32, tag=f"ps{n}", name=f"ps{b}_{n}")
                    for n in range(2)
                ]
                for c in range(CHUNKS):
                    ht = hp.tile([P, HID], sb_dt, tag="h")
                    if mode == "f32":
                        nc.sync.dma_start(ht[:], h4[b, c])
                    else:
                        nc.gpsimd.dma_start(ht[:], h4[b, c])
                    for n in range(2):
                        nc.tensor.matmul(
                            ps[n][:],
                            wts[b][:, c, :],
                            ht[:, n * 512 : (n + 1) * 512],
                            start=(c == 0),
                            stop=(c == CHUNKS - 1),
                        )
                ot = op.tile([NS, HID], mybir.dt.float32, tag="o")
                for n in range(2):
                    nc.vector.tensor_copy(ot[:, n * 512 : (n + 1) * 512], ps[n][:])
                nc.sync.dma_start(o.ap()[b], ot[:])
    nc.compile()
    return nc


def _get_nc(mode=None):
    mode = mode or MODE
    if mode not in _nc_cache:
        _nc_cache[mode] = _build_nc(mode)
    return _nc_cache[mode]


def _prepare(last_hidden_state, sentence_mask, num_sents):
    """Host prep: shard on batch, build the scaled one-hot mask tensor."""
    lhs = np.ascontiguousarray(np.asarray(last_hidden_state, dtype=np.float32))
    mask = np.asarray(sentence_mask)
    ns = int(num_sents)
    assert lhs.shape == (BATCH, SEQ, HID) and ns == NS, (lhs.shape, ns)

    ids = mask.astype(np.int64)
    onehot = ids[:, :, None] == np.arange(ns, dtype=np.int64)[None, None, :]
    counts = onehot.sum(axis=1)  # [B, NS]
    scale = 1.0 / np.maximum(counts, 1).astype(np.float64)
    w = onehot.astype(np.float32) * scale[:, None, :].astype(np.float32)
    # [B, S, NS] -> [B, P, CHUNKS, NS] with S = c*P + p
    w = np.ascontiguousarray(
        w.reshape(BATCH, CHUNKS, P, ns).transpose(0, 2, 1, 3)
    )

    in_maps = [
        {
            "h": lhs[i * BPC : (i + 1) * BPC],
            "w": w[i * BPC : (i + 1) * BPC],
        }
        for i in range(NCORES)
    ]
    return in_maps, mask.dtype


def _execute(in_maps, trace=False, mode=None, **kwargs):
    from concourse.bass_utils import run_bass_kernel_spmd

    return run_bass_kernel_spmd(
        _get_nc(mode),
        in_maps,
        core_ids=list(range(NCORES)),
        trace=trace,
        **kwargs,
    )


def _gather(results):
    emb = np.concatenate([r["o"] for r in results], axis=0)
    return emb


def kernel(last_hidden_state, sentence_mask, num_sents):
    in_maps, mask_dtype = _prepare(last_hidden_state, sentence_mask, num_sents)
    res = _execute(in_maps)
    emb = _gather(res.results)
    unique_sents = np.arange(int(num_sents), dtype=mask_dtype)
    return emb, unique_sents
